# revision 35
# baseline (speedup 1.0000x reference)
"""Trainium2 Bass kernel for nn_CLLayer_47064251630125 (contrastive loss).

Reference computation (per row i of N=8192):
    h1 = ELU(z1 @ W1.T + b1) @ W2.T + b2 ; h2 likewise
    na = normalize(h1), nb = normalize(h2)   (L2 row norm)
    l1 = -log( exp(2 na_i.nb_i) / (sum_j exp(2 na_i.na_j) + sum_j exp(2 na_i.nb_j) - e^2) )
    l2 = same with roles swapped (uses column sums of the cross matrix)
    out = (l1 + l2)/2

Sharding (v2, data-parallel + AllGather): each core receives ONLY its
1024-row block of z1/z2 (transposed, bf16) -> 8.5MB total host->device
staging instead of 77.6MB replicated.  Per core:
  1. project + ELU its block (PE), L2-normalize via PE-ones column sums
     + Ln/Exp, quantize the normalized projection to fp8e4 (x16 scale).
  2. AllGather the fp8 blocks (256KB in -> 2MB out, DRAM), one gather
     per tensor, overlapped with the other tensor's projection / the
     first similarity stream.
  3. three [block x full] similarity streams off the gathered fp8 with
     DoubleRow fp8 matmuls (2x PE) and fused exp+row-sum on ACT:
       R1 = (na, na)  -> denom1 refl term
       B  = (na, nb)  -> denom1 cross term; exp tiles column-summed
                         (PE ones) and ReduceScatter'ed for denom2
       R2 = (nb, nb)  -> denom2 refl term
  4. the positive-pair term uses the full-precision local h blocks:
     pos = (h1_i.h2_i) * rn1_i * rn2_i; l = 0.5*(ln d1 + ln d2) - 2*pos.
"""

import sys

sys.path.insert(0, "/opt/trn_rl_repo")

import numpy as np
import ml_dtypes

import concourse.bass as bass
import concourse.mybir as mybir
import concourse.tile as tile
from concourse import bacc

BF16 = mybir.dt.bfloat16
F32 = mybir.dt.float32
F8 = mybir.dt.float8e4
AF = mybir.ActivationFunctionType
ALU = mybir.AluOpType
DR = mybir.MatmulPerfMode.DoubleRow

P = 128
D = 256
KT = D // P          # 2 k-tiles
N_FULL = 8192
N_CORES = 8
CH = 512
TAU = 0.5
SIM_SCALE = 1.0 / TAU          # 2.0
QS = 16.0                      # fp8 quantization scale per side
EXP_SCALE = SIM_SCALE / (QS * QS)
E2 = float(np.exp(SIM_SCALE))  # exp(2 * ||na||^2), diag of refl


def build_bass(n_full=N_FULL, blk=None, n_cores=N_CORES):
    """Trace the Tile kernel.  Returns the compiled Bacc object (SPMD)."""
    if blk is None:
        blk = n_full // n_cores
    NCH = blk // CH               # projection chunks per tensor
    ISUB = blk // P               # i-subtiles per core block
    JS = blk // CH                # 512-wide col tiles per rank slot
    NT = n_full // CH             # total 512-wide col tiles
    RG = min(4, NT)               # col tiles per R1/R2 exp group (2048 wide)
    NG = NT // RG                 # R1/R2 groups per isub

    nc = bacc.Bacc("TRN2", target_bir_lowering=False, debug=False,
                   num_devices=n_cores)

    z1b = nc.dram_tensor("z1b", [D, blk], F8, kind="ExternalInput")
    z2b = nc.dram_tensor("z2b", [D, blk], F8, kind="ExternalInput")
    w1t = nc.dram_tensor("w1t", [D, D], BF16, kind="ExternalInput")
    w2t = nc.dram_tensor("w2t", [D, D], BF16, kind="ExternalInput")
    b1d = nc.dram_tensor("b1", [D], F32, kind="ExternalInput")
    b2d = nc.dram_tensor("b2", [D], F32, kind="ExternalInput")
    out = nc.dram_tensor("out", [P, ISUB], F32, kind="ExternalOutput")

    ag_in = [nc.dram_tensor(f"ag_in{i}", [KT, P, blk], F8, kind="Internal")
             for i in range(2)]
    ag_space = "Shared" if n_cores > 4 else "Local"
    ag_out = [nc.dram_tensor(f"ag_out{i}", [n_cores, KT, P, blk], F8,
                             kind="Internal", addr_space=ag_space)
              for i in range(2)]
    rr_d = nc.dram_tensor("rr_d", [blk], F32, kind="Internal")
    cc_in = nc.dram_tensor("cc_in", [n_full], F32, kind="Internal")
    cc_out = nc.dram_tensor("cc_out", [blk], F32, kind="Internal")

    groups = [list(range(n_cores))]

    with tile.TileContext(nc) as tc:
        with (
            tc.tile_pool(name="const", bufs=1) as cpool,
            tc.tile_pool(name="persist", bufs=1) as ppool,
            tc.tile_pool(name="io", bufs=4) as iopool,
            tc.tile_pool(name="scratch", bufs=4) as spool,
        ):
            # ---- constants ----
            w1_sb = cpool.tile([P, KT, D], BF16)
            nc.sync.dma_start(w1_sb, w1t.rearrange("(k p) c -> p k c", p=P))
            w2_sb = cpool.tile([P, KT, D], BF16)
            nc.sync.dma_start(w2_sb, w2t.rearrange("(k p) c -> p k c", p=P))
            b1f = cpool.tile([P, KT], F32)
            nc.sync.dma_start(b1f, b1d.rearrange("(m p) -> p m", p=P))
            b2f = cpool.tile([P, KT], F32)
            nc.sync.dma_start(b2f, b2d.rearrange("(m p) -> p m", p=P))
            # derived bias forms for the relu path: relu(x+b) = max(x,-b)+b
            nb1 = cpool.tile([P, KT], F32)
            nc.vector.tensor_scalar_mul(nb1, b1f, -1.0)
            b1p1 = cpool.tile([P, KT], F32)
            nc.vector.tensor_scalar_add(b1p1, b1f, 1.0)
            ones_col = cpool.tile([P, 1], BF16)
            nc.vector.memset(ones_col, 1.0)
            ones_row = cpool.tile([1, P], F32)
            nc.vector.memset(ones_row, 1.0)
            ln16 = cpool.tile([1, 1], F32)
            nc.vector.memset(ln16, float(np.log(QS)))
            # prefetch the natural_log_exp ACT table set under input DMAs:
            # Ln first narrows the possible sets to {natural_log,
            # natural_log_exp}; the first Exp then resolves to
            # natural_log_exp with no further table swaps.
            warm = cpool.tile([P, 1], BF16)
            nc.scalar.activation(warm, ones_col, AF.Ln)
            nc.scalar.activation(warm, ones_col, AF.Exp)

            rs = ppool.tile([P, 3, ISUB * max(NT, 1)], F32)
            nc.vector.memset(rs, 0.0)
            fin = ppool.tile([P, 10, ISUB], F32)

            naq = []   # local fp8 normalized blocks [P, KT, blk]
            naF = []   # gathered fp8 full tensors [P, n_cores, KT, blk]
            hTs = []
            rn_alls = []

            # ================= projection of z1 then z2 =================
            for idx, zb in enumerate((z1b, z2b)):
                zb_ap = zb.rearrange("(k p) w -> p k w", p=P)
                hT = ppool.tile([P, KT, blk], BF16, name=f"hT{idx}")
                nq = ppool.tile([P, KT, blk], F8, name=f"naq{idx}")

                with tc.tile_pool(name=f"psA{idx}", bufs=2, space="PSUM") \
                        as psA:
                    ns_all = psA.tile([1, NCH, CH], F32, name="ns",
                                      tag="ns", bufs=1)
                    for c in range(NCH):
                        cs = slice(c * CH, (c + 1) * CH)
                        zch8 = iopool.tile([P, KT, CH], F8, tag="zch8")
                        nc.sync.dma_start(zch8, zb_ap[:, :, cs])
                        zch = iopool.tile([P, KT, CH], BF16, tag="zch")
                        nc.vector.tensor_copy(zch, zch8)
                        # L1: pa[m] = W1 @ z.T  (biases in the epilogues)
                        pa = psA.tile([P, KT, CH], F32, name="pa", tag="pa",
                                      bufs=2)
                        for m in range(KT):
                            ms = slice(m * P, (m + 1) * P)
                            for k in range(KT):
                                nc.tensor.matmul(pa[:, m], w1_sb[:, k, ms],
                                                 zch[:, k], start=(k == 0),
                                                 stop=(k == KT - 1))
                        # ELU' = elu+1 = min(exp(x+b1), relu(x+b1)+1)
                        # (the -1 is folded into b2 on the host)
                        e_t = spool.tile([P, KT, CH], BF16, tag="e")
                        r_t = spool.tile([P, KT, CH], BF16, tag="r")
                        aT = spool.tile([P, KT, CH], BF16, tag="aT")
                        for m in range(KT):
                            nc.scalar.activation(e_t[:, m], pa[:, m], AF.Exp,
                                                 bias=b1f[:, m:m + 1])
                            nc.vector.tensor_scalar(r_t[:, m], pa[:, m],
                                                    nb1[:, m:m + 1],
                                                    b1p1[:, m:m + 1],
                                                    ALU.max, ALU.add)
                            nc.vector.tensor_tensor(aT[:, m], e_t[:, m],
                                                    r_t[:, m], ALU.min)
                        # L2: ph[m2] = W2 @ a
                        ph = psA.tile([P, KT, CH], F32, name="ph", tag="ph",
                                      bufs=1)
                        for m2 in range(KT):
                            ms = slice(m2 * P, (m2 + 1) * P)
                            for m in range(KT):
                                nc.tensor.matmul(ph[:, m2], w2_sb[:, m, ms],
                                                 aT[:, m], start=(m == 0),
                                                 stop=(m == KT - 1))
                        # h = ph + b2 -> sbuf bf16; sq = h*h; norms on PE
                        sq = spool.tile([P, KT, CH], BF16, tag="sq")
                        for m2 in range(KT):
                            nc.vector.tensor_scalar(hT[:, m2, cs],
                                                    ph[:, m2],
                                                    b2f[:, m2:m2 + 1],
                                                    None, ALU.add)
                            nc.vector.tensor_tensor(sq[:, m2], hT[:, m2, cs],
                                                    hT[:, m2, cs], ALU.mult)
                        for m2 in range(KT):
                            nc.tensor.matmul(ns_all[:, c], ones_col,
                                             sq[:, m2], start=(m2 == 0),
                                             stop=(m2 == KT - 1))
                    # rn = QS * ns^-1/2 = exp(-0.5 ln ns + ln QS), one
                    # Ln/Exp per tensor (avoids ACT table thrash), then
                    # PE-broadcast per chunk and quantize to fp8
                    lns = spool.tile([1, NCH, CH], F32, tag="lns")
                    nc.scalar.activation(lns, ns_all, AF.Ln)
                    rn_all = ppool.tile([1, NCH, CH], F32,
                                        name=f"rn{idx}", tag=f"rn{idx}")
                    nc.scalar.activation(rn_all, lns, AF.Exp, scale=-0.5,
                                         bias=ln16)
                    rn_alls.append(rn_all)
                    for c in range(NCH):
                        cs = slice(c * CH, (c + 1) * CH)
                        rnB = psA.tile([P, CH], F32, name="rnB", tag="ph",
                                       bufs=1)
                        nc.tensor.matmul(rnB, ones_row, rn_all[0:1, c],
                                         start=True, stop=True)
                        for k in range(KT):
                            nc.vector.tensor_tensor(nq[:, k, cs],
                                                    hT[:, k, cs],
                                                    rnB, ALU.mult)
                nc.sync.dma_start(ag_in[idx].rearrange("k p j -> p k j"), nq)
                nc.gpsimd.collective_compute(
                    "AllGather", ALU.bypass, replica_groups=groups,
                    ins=[ag_in[idx][:]], outs=[ag_out[idx][:]])
                nF = ppool.tile([P, n_cores, KT, blk], F8, name=f"naF{idx}")
                nc.sync.dma_start(nF, ag_out[idx].rearrange(
                    "r k p j -> p r k j"))
                naq.append(nq)
                naF.append(nF)
                hTs.append(hT)

            # rr = rn1 * rn2 / QS^2 (per column), roundtrip to [P, ISUB]
            rr_sb = spool.tile([1, NCH, CH], F32, tag="rr_sb")
            nc.vector.scalar_tensor_tensor(rr_sb, rn_alls[0],
                                           1.0 / (QS * QS), rn_alls[1],
                                           ALU.mult, ALU.mult)
            nc.sync.dma_start(rr_d[None, :], rr_sb[0:1, :, :])
            rrv = ppool.tile([P, ISUB], F32)
            nc.sync.dma_start(rrv, rr_d.rearrange("(s p) -> p s", p=P))

            # Schraudolph-style bf16 exp for the DVE path:
            # bf16 bits of exp(EXP_SCALE*x) ~ x*EXP_SCALE*log2(e)*128 + 16250
            PWL_C1 = float(EXP_SCALE * np.log2(np.e) * 128.0)
            PWL_C2 = 16250.0
            I16 = mybir.dt.int16

            def emit_group(st, lhs_q, rhs_F, tiles, gi, isub, psS, nbufs,
                           eg_tag, dve=False, pool_reduce=False):
                """One [128 x len(tiles)*CH] similarity group: fp8 DoubleRow
                matmuls + exp/row-sum, on ACT (table exp + accum_out) or on
                DVE (bit-trick exp; row-sum on DVE or Pool)."""
                nt = len(tiles)
                lhs = lhs_q[:, :, isub * P:(isub + 1) * P]
                pg = psS.tile([P, nt, CH], F32, tag="sgrp", bufs=nbufs,
                              name="pg")
                for g, (r, js) in enumerate(tiles):
                    jss = slice(js * CH, (js + 1) * CH)
                    nc.tensor.matmul(pg[:, g], lhs, rhs_F[:, r, :, jss],
                                     start=True, stop=True, perf_mode=DR)
                eg = spool.tile([P, nt, CH], BF16,
                                tag=(eg_tag + "p") if dve else eg_tag,
                                bufs=4)
                if dve:
                    eng = nc.gpsimd if pool_reduce else nc.vector
                    eng.tensor_scalar(
                        eg[:, :, :].bitcast(I16), pg, PWL_C1, PWL_C2,
                        ALU.mult, ALU.add)
                    nc.vector.tensor_reduce(
                        out=rs[:, st, gi:gi + 1, None], in_=eg, op=ALU.add,
                        axis=mybir.AxisListType.XY)
                else:
                    nc.scalar.activation(eg, pg, AF.Exp, scale=EXP_SCALE,
                                         accum_out=rs[:, st, gi:gi + 1])
                return eg

            all_tiles = [(r, js) for r in range(n_cores) for js in range(JS)]

            # ---- stream R1 = (na1, na1): overlaps the z2 gather ----
            with tc.tile_pool(name="psR1", bufs=1, space="PSUM") as psS:
                for isub in range(ISUB):
                    for a in range(NG):
                        gi = isub * NG + a
                        emit_group(0, naq[0], naF[0],
                                   all_tiles[a * RG:(a + 1) * RG],
                                   gi, isub, psS, 2, "egd",
                                   dve=(gi % 11 == 5))

            # ---- pos + stream B = (na1, na2) with column sums + RS ----
            with tc.tile_pool(name="psB", bufs=1, space="PSUM") as psB:
                pd = spool.tile([P, KT, blk], BF16, tag="pd", bufs=1)
                for k in range(KT):
                    nc.vector.tensor_tensor(pd[:, k], hTs[0][:, k],
                                            hTs[1][:, k], ALU.mult)
                pos_ps = psB.tile([P, ISUB], F32, name="pos_ps", bufs=3,
                                  tag="sgrp")
                for s in range(ISUB):
                    ss = slice(s * P, (s + 1) * P)
                    for k in range(KT):
                        nc.tensor.matmul(pos_ps[:, s:s + 1], pd[:, k, ss],
                                         ones_col, start=(k == 0),
                                         stop=(k == KT - 1))
                nc.vector.tensor_tensor(fin[:, 9], pos_ps, rrv, ALU.mult)

                cs_tiles = [psB.tile([1, CH], F32, name=f"cst{js}",
                                     tag=f"cst{js}", bufs=1)
                            for js in range(JS)]
                for r in range(n_cores):
                    for isub in range(ISUB):
                        eg = emit_group(1, naq[0], naF[1],
                                        [(r, js) for js in range(JS)],
                                        isub * n_cores + r, isub, psB, 3,
                                        "eg", dve=((isub + r) % 3 == 1))
                        for js in range(JS):
                            nc.tensor.matmul(cs_tiles[js], ones_col,
                                             eg[:, js], start=(isub == 0),
                                             stop=(isub == ISUB - 1))
                    for js in range(JS):
                        cs_sb = spool.tile([1, CH], F32, tag="cs_sb")
                        nc.vector.tensor_copy(cs_sb, cs_tiles[js])
                        joff = r * blk + js * CH
                        nc.sync.dma_start(cc_in[None, joff:joff + CH],
                                          cs_sb[0:1, :])
                nc.gpsimd.collective_compute(
                    "ReduceScatter", ALU.add, replica_groups=groups,
                    ins=[cc_in[:]], outs=[cc_out[:]])

            # d1-side finals overlap stream R2 (their inputs are ready)
            ccv = ppool.tile([P, ISUB], F32)
            nc.sync.dma_start(ccv, cc_out.rearrange("(s p) -> p s", p=P))
            r1v = rs[:, 0, :ISUB * NG].rearrange("p (i g) -> p i g", g=NG)
            bv = rs[:, 1, :ISUB * n_cores].rearrange("p (i g) -> p i g",
                                                     g=n_cores)
            r2v = rs[:, 2, :ISUB * NG].rearrange("p (i g) -> p i g", g=NG)
            nc.vector.tensor_reduce(out=fin[:, 0, :, None], in_=r1v,
                                    op=ALU.add, axis=mybir.AxisListType.X)
            nc.vector.tensor_reduce(out=fin[:, 1, :, None], in_=bv,
                                    op=ALU.add, axis=mybir.AxisListType.X)
            nc.vector.scalar_tensor_tensor(fin[:, 4], fin[:, 0], -E2,
                                           fin[:, 1], ALU.add, ALU.add)
            nc.scalar.activation(fin[:, 6], fin[:, 4], AF.Ln)

            # ---- stream R2 = (na2, na2), split across ACT and DVE ----
            with tc.tile_pool(name="psR2", bufs=1, space="PSUM") as psS:
                for isub in range(ISUB):
                    for a in range(NG):
                        emit_group(2, naq[1], naF[1],
                                   all_tiles[a * RG:(a + 1) * RG],
                                   isub * NG + a, isub, psS, 2, "egd",
                                   dve=((isub * NG + a) % 3 == 1))

            # ---- final: l = 0.5*(ln d1 + ln d2 - 2*2*pos) ----
            nc.vector.tensor_reduce(out=fin[:, 3, :, None], in_=r2v,
                                    op=ALU.add, axis=mybir.AxisListType.X)
            nc.vector.scalar_tensor_tensor(fin[:, 5], fin[:, 3], -E2,
                                           ccv, ALU.add, ALU.add)
            nc.scalar.activation(fin[:, 7], fin[:, 5], AF.Ln)
            nc.vector.tensor_tensor(fin[:, 8], fin[:, 6], fin[:, 7], ALU.add)
            lres = ppool.tile([P, ISUB], F32)
            nc.vector.scalar_tensor_tensor(lres, fin[:, 9],
                                           -2.0 * SIM_SCALE, fin[:, 8],
                                           ALU.mult, ALU.add)
            nc.vector.tensor_scalar_mul(lres, lres, 0.5)
            nc.sync.dma_start(out[:, :], lres)

    nc.compile()
    return nc


def prep_inputs(z1, z2, W1, b1, W2, b2, n_full=N_FULL, n_cores=N_CORES):
    """Host-side prep -> list of per-core input maps (numpy)."""
    blk = n_full // n_cores
    bf = ml_dtypes.bfloat16
    f8 = mybir.dt.np(mybir.dt.float8e4)
    z1t = np.ascontiguousarray(z1.T)
    z2t = np.ascontiguousarray(z2.T)
    w1t = np.ascontiguousarray(W1.T).astype(bf)
    w2t = np.ascontiguousarray(W2.T).astype(bf)
    # ELU' = elu + 1 is used as the L1 activation; fold the "-1" into b2:
    # h = W2 @ (elu'(x) - 1) + b2 = W2 @ elu'(x) + (b2 - W2.sum(1))
    b2_eff = (b2 - W2.sum(axis=1)).astype(np.float32)
    b1c = b1.astype(np.float32)
    in_maps = []
    for c in range(n_cores):
        bs = slice(c * blk, (c + 1) * blk)
        in_maps.append({
            "z1b": np.ascontiguousarray(z1t[:, bs]).astype(f8),
            "z2b": np.ascontiguousarray(z2t[:, bs]).astype(f8),
            "w1t": w1t, "w2t": w2t, "b1": b1c, "b2": b2_eff,
        })
    return in_maps


_NC_CACHE = {}


def _get_nc(n_full=N_FULL, n_cores=N_CORES):
    key = (n_full, n_cores)
    if key not in _NC_CACHE:
        _NC_CACHE[key] = build_bass(n_full=n_full, n_cores=n_cores)
    return _NC_CACHE[key]


def kernel(z1, z2, W1, b1, W2, b2):
    from concourse.bass_utils import run_bass_kernel_spmd

    n_full = z1.shape[0]
    n_cores = N_CORES
    in_maps = prep_inputs(z1, z2, W1, b1, W2, b2, n_full, n_cores)
    nc = _get_nc(n_full, n_cores)
    res = run_bass_kernel_spmd(nc, in_maps, core_ids=list(range(n_cores)))
    parts = [np.asarray(res.results[c]["out"]).T.reshape(-1)
             for c in range(n_cores)]
    return np.concatenate(parts).astype(np.float32)


if __name__ == "__main__":
    nc = build_bass()
    print("traced ok")
